# revision 60
# baseline (speedup 1.0000x reference)
"""Causal multi-head attention forward (B=2, T=2048, C=1024, H=16, D=64)
for 8 Trainium2 NeuronCores.

Sharding: core = (batch b, head-group hg) with b in {0,1}, hg in {0..3};
each core computes QKV projection for its 4 heads on its batch, causal
flash attention for those heads, and a partial output projection
(contraction over its 256 head-feature rows of W_o). Host sums the 4
partials per batch (f32) and adds b_o (+ folded b_v @ W_o).

Key layout decisions (per core):
  - All matmuls run with K=128 (PE stays in one tile config; switching
    between 64-row and 128-row configs costs ~130 ns per switch on HW).
    S = k^T q uses per-head stationary tiles kT[h] [128, T] where the
    64 rows matching the head's position in the packed q tile hold k
    and the other 64 rows are zeroed once at startup.
  - v_all groups [ones(64) | v(64)] per (t-block, head): the PV matmul
    emits the softmax denominator l duplicated on PSUM partitions 0..63
    and y^T on 64..127, so normalize is recip([64,TQ]) + one multiply
    (no partition broadcast).
  - Stages are interleaved: proj(0), attn(0), proj(1), oproj(0),
    attn(1), proj(2), oproj(1), attn(2), proj(3), oproj(2), attn(3),
    oproj(3). ScalarE exp overlaps projection/o_proj PE work, and the
    proj between attn(i) and oproj(i) hides the normalize latency.
  - Diagonal S blocks keep dead-column prefixes in persistent es tiles
    zeroed once at startup (exp writes live columns only); the diagonal
    128x128 square is masked by a DVE multiply with a host-provided
    triangular mask.
  - o_proj partials leave the chip as bf16 (host accumulates in f32),
    halving output DMA bytes.
"""

import os
import sys
from contextlib import ExitStack
from dataclasses import dataclass

import numpy as np

for _p in ("/opt/trn_rl_repo",):
    if _p not in sys.path and os.path.isdir(_p):
        sys.path.insert(0, _p)

import ml_dtypes

import concourse.bass as bass
import concourse.bacc as bacc
import concourse.mybir as mybir
import concourse.tile as tile


def _install_axon_ntff_hook():
    """Provide antenv.axon_hooks (absent on this image) so bass_utils'
    trace path works; registers the ctypes NTFF hook when available."""
    import types

    if "antenv.axon_hooks" not in sys.modules:
        import antenv

        mod = types.ModuleType("antenv.axon_hooks")
        _reg = [None]
        mod.get_axon_ntff_profile_hook = lambda: _reg[0]
        mod.set_axon_ntff_profile_hook = lambda h: _reg.__setitem__(0, h)
        sys.modules["antenv.axon_hooks"] = mod
        antenv.axon_hooks = mod
    hooks = sys.modules["antenv.axon_hooks"]
    if hooks.get_axon_ntff_profile_hook() is not None:
        return
    try:
        import contextlib
        import ctypes

        lib = ctypes.CDLL("/opt/axon/libaxon_pjrt.so")
        if not hasattr(lib, "axon_start_nrt_profile"):
            return
        lib.axon_start_nrt_profile.argtypes = [
            ctypes.POINTER(ctypes.c_int64), ctypes.c_size_t]
        lib.axon_start_nrt_profile.restype = ctypes.c_int64
        lib.axon_stop_nrt_profile.argtypes = [ctypes.c_char_p]
        lib.axon_stop_nrt_profile.restype = ctypes.c_int64

        @contextlib.contextmanager
        def _hook(output_dir, device_ids):
            import jax

            jax.devices()
            if device_ids:
                ids = (ctypes.c_int64 * len(device_ids))(*device_ids)
                rc = lib.axon_start_nrt_profile(ids, len(device_ids))
            else:
                rc = lib.axon_start_nrt_profile(None, 0)
            if rc != 0:
                raise RuntimeError(f"axon_start_nrt_profile rc={rc}")
            try:
                yield
            finally:
                n = lib.axon_stop_nrt_profile(str(output_dir).encode())
                print(f"ntff profile: {n} file(s) -> {output_dir}",
                      file=sys.stderr)

        hooks.set_axon_ntff_profile_hook(_hook)
    except Exception:
        pass


try:
    _install_axon_ntff_hook()
except Exception:
    pass

BF16 = mybir.dt.bfloat16
F32 = mybir.dt.float32
AF = mybir.ActivationFunctionType
ALU = mybir.AluOpType
NPBF16 = ml_dtypes.bfloat16

P = 128


@dataclass(frozen=True)
class Cfg:
    T: int = 2048  # sequence length
    C: int = 1024  # input feature dim
    CO: int = 1024  # output feature dim (W_o cols)
    D: int = 64  # head dim
    HL: int = 4  # local heads per core (2 row-packed pairs)
    TQ: int = 512  # query-chunk size

    @property
    def CB(self):  # c blocks
        return self.C // P

    @property
    def NFB(self):  # qk f-blocks (q+k for HL heads)
        return 2 * self.HL * self.D // P

    @property
    def NQC(self):  # query chunks
        return self.T // self.TQ

    @property
    def TCB(self):  # t blocks of 128 (ki blocks / o_proj rows)
        return self.T // P

    @property
    def VG(self):  # v group width: [ones(64) | v(64)]
        return self.D + 64


def emit_kernel(tc: tile.TileContext, cfg: Cfg, ins: dict, out_ap: bass.AP,
                ctx: ExitStack):
    nc = tc.nc
    T, C, CO, D, HL, TQ = cfg.T, cfg.C, cfg.CO, cfg.D, cfg.HL, cfg.TQ
    VG = cfg.VG
    NQC, CB = cfg.NQC, cfg.CB
    assert HL == 4 and D == 64 and TQ == 512

    io = ctx.enter_context(tc.tile_pool(name="io", bufs=1))

    # ---- persistent SBUF tiles + input DMA (priority order) ----
    tri_sb = io.tile([P, P], BF16, name="tri", tag="tri")
    nc.sync.dma_start(tri_sb, ins["tri"][:, :])  # first: warmup reads it
    bscale_sb = io.tile([P, cfg.NFB], F32, name="bscale", tag="bscale")
    nc.sync.dma_start(bscale_sb, ins["bscale"][:, :])
    bbias_sb = io.tile([P, cfg.NFB], F32, name="bbias", tag="bbias")
    nc.sync.dma_start(bbias_sb, ins["bbias"][:, :])
    # x^T in per-(cb, chunk) tiles; DRAM layout is chunk-major so every
    # chunk DMA is one fully-contiguous 128KB block
    xTc = [[io.tile([P, TQ], BF16, name=f"xT{cb}_{tq}", tag=f"xT{cb}_{tq}")
            for tq in range(NQC)] for cb in range(CB)]

    def dma_x_chunk(tq):
        for cb in range(CB):
            r0 = (tq * CB + cb) * P
            # chunk 0 is the priming transfer: split across SP + GpSimd
            # queues (GpSimd drains long before the end of the kernel)
            eng = nc.gpsimd if tq == 0 and cb >= CB // 2 else nc.sync
            eng.dma_start(xTc[cb][tq], ins["xT"][r0:r0 + P, :])

    # weights ride the second DMA queue (Activation HWDGE) in parallel
    # with x chunks on the SP queue — halves the start-up priming time.
    # ScalarE is idle until ~15us, so the queue occupancy is free.
    wqk_sb = []
    for cb in range(CB):
        wq = io.tile([P, 2 * HL * D], BF16, name=f"wqk{cb}", tag=f"wqk{cb}")
        eng = nc.scalar if cb < CB // 2 else nc.gpsimd
        eng.dma_start(wq, ins["wqk"][cb * P:(cb + 1) * P, :])
        wqk_sb.append(wq)
    dma_x_chunk(0)
    wv_sb = []
    for cb in range(CB):
        wvt = io.tile([P, HL * D], BF16, name=f"wv{cb}", tag=f"wv{cb}")
        nc.scalar.dma_start(wvt, ins["wv"][cb * P:(cb + 1) * P, :])
        wv_sb.append(wvt)
    dma_x_chunk(1)
    wo_sb = []
    for fb in range(HL * D // P):
        wot = io.tile([P, CO], BF16, name=f"wo{fb}", tag=f"wo{fb}")
        nc.sync.dma_start(wot, ins["wo"][fb * P:(fb + 1) * P, :])
        wo_sb.append(wot)
    dma_x_chunk(2)
    dma_x_chunk(3)

    # ---- persistent compute tiles ----
    qT_sb = [io.tile([P, T], BF16, name=f"qT{hp}", tag=f"qT{hp}")
             for hp in range(2)]
    # per-head k, zero-padded in the 64 rows not matching the head's slot
    # in the packed q tile (h even -> k at rows 0:64, h odd -> rows 64:128)
    kT_sb = [io.tile([P, T], BF16, name=f"kT{h}", tag=f"kT{h}")
             for h in range(HL)]
    v_all = io.tile([P, cfg.TCB * HL * VG], BF16, name="v_all", tag="v_all")
    yT_sb = [io.tile([P, T], BF16, name=f"yT{hp}", tag=f"yT{hp}")
             for hp in range(2)]
    rT = [io.tile([D, TQ], F32, name=f"rT{h2}", tag=f"rT{h2}")
          for h2 in range(2)]

    # ---- startup memsets, split across DVE and GpSimd ----
    nc.vector.memset(kT_sb[0][64:P, :], 0.0)
    nc.vector.memset(kT_sb[1][0:64, :], 0.0)
    nc.gpsimd.memset(kT_sb[2][64:P, :], 0.0)
    nc.gpsimd.memset(kT_sb[3][0:64, :], 0.0)
    # ones block of every v group (-> l duplicated on PSUM rows 0..63)
    ones_view = v_all.rearrange("p (g c) -> p g c", c=VG)[:, :, 0:D]
    nc.gpsimd.memset(ones_view, 1.0)

    with (tc.tile_pool(name="psh", bufs=2, space="PSUM") as psh,
          tc.tile_pool(name="psS", bufs=2, space="PSUM") as psS,
          tc.tile_pool(name="psY", bufs=1, space="PSUM") as psY,
          tc.tile_pool(name="esb", bufs=6) as esb,
          tc.tile_pool(name="osb", bufs=3) as osb):

        def emit_proj(tq):
            # q/k projection chains for this column chunk. Chains 2 and 3
            # borrow the (idle-between-attentions) psY banks so the first
            # chains never wait on the previous stage's eviction backlog.
            for fb in range(cfg.NFB):
                if tq > 0 and fb in (2, 3):
                    ps = psY.tile([P, TQ], F32, tag=f"y{fb - 2}",
                                  name="ps_qk")
                else:
                    ps = psh.tile([P, TQ], F32, tag="sh", name="ps_qk")
                for cb in range(CB):
                    nc.tensor.matmul(
                        ps, wqk_sb[cb][:, fb * P:(fb + 1) * P], xTc[cb][tq],
                        start=(cb == 0), stop=(cb == CB - 1))
                # evictions on DVE (tensor_scalar affine) — ScalarE is the
                # exp pacer and must not carry eviction work
                if fb < 2:  # q, packed 2 heads
                    nc.vector.tensor_scalar(
                        qT_sb[fb][:, tq * TQ:(tq + 1) * TQ], ps,
                        bscale_sb[:, fb:fb + 1], bbias_sb[:, fb:fb + 1],
                        op0=ALU.mult, op1=ALU.add)
                else:  # k, split per head into zero-padded tiles
                    for half in range(2):
                        h = (fb - 2) * 2 + half
                        r0, r1 = half * D, (half + 1) * D
                        nc.vector.tensor_scalar(
                            kT_sb[h][r0:r1, tq * TQ:(tq + 1) * TQ],
                            ps[r0:r1, :],
                            bscale_sb[r0:r1, fb:fb + 1],
                            bbias_sb[r0:r1, fb:fb + 1],
                            op0=ALU.mult, op1=ALU.add)
            # v projection: two t-blocks share one PSUM tile, evict on DVE
            for t2 in range(2):
                psv = psh.tile([P, TQ], F32, tag="sh", name="ps_v")
                for j in range(2):
                    for cb in range(CB):
                        nc.tensor.matmul(
                            psv[:, j * HL * D:(j + 1) * HL * D],
                            xTc[cb][tq][:, (t2 * 2 + j) * P:(t2 * 2 + j + 1) * P],
                            wv_sb[cb],
                            start=(cb == 0), stop=(cb == CB - 1))
                for j in range(2):
                    tb = tq * 4 + t2 * 2 + j
                    vdst = v_all[:, tb * HL * VG:(tb + 1) * HL * VG]
                    vdst = vdst.rearrange("p (h g) -> p h g", g=VG)[:, :, D:VG]
                    nc.vector.tensor_copy(
                        vdst,
                        psv[:, j * HL * D:(j + 1) * HL * D].rearrange(
                            "p (h d) -> p h d", d=D))

        def emit_attn(qc, fillers=()):
            # Block-granularity software pipeline: S(kb) overlaps exp(kb-1)
            # and PV(kb-1) (sps double-buffered per h2, es pool depth 3).
            # Diagonal blocks compute/exp/accumulate live columns only.
            # fillers: closures each emitting one o_proj chain of the
            # previous chunk, spread out to absorb the exp-latency slack.
            fillers = list(fillers)
            nkb = (qc + 1) * TQ // P
            f_i = 0
            blk_i = 0
            nblk = 2 * nkb
            for hp in range(2):
                yps = [psY.tile([P, TQ], F32, tag=f"y{h2}", name=f"ps_y{h2}")
                       for h2 in range(2)]

                def emit_pv(kb, jj, es, norm):
                    for h2 in range(2):
                        h = hp * 2 + h2
                        nc.tensor.matmul(
                            yps[h2][:, jj:TQ],
                            v_all[:, (kb * HL + h) * VG:(kb * HL + h + 1) * VG],
                            es[:, h2 * TQ + jj:(h2 + 1) * TQ],
                            start=(kb == 0), stop=(kb == nkb - 1),
                            skip_group_check=True)
                        if norm:  # l duplicated on PSUM rows 0..63
                            nc.vector.reciprocal_approx_fast(
                                rT[h2], yps[h2][0:D, :])
                            nc.vector.tensor_tensor(
                                yT_sb[hp][h2 * D:(h2 + 1) * D,
                                          qc * TQ:(qc + 1) * TQ],
                                yps[h2][D:2 * D, :], rT[h2], op=ALU.mult)

                # 2-block PV lag: exp(kb) (one merged instruction) has a
                # full extra PE block of slack before PV(kb) needs it, so
                # the PE never waits on ScalarE in steady state.
                pending = []
                for kb in range(nkb):
                    jj = max(0, kb * P - qc * TQ)
                    diag = kb * P >= qc * TQ
                    # one [128, 2*TQ] tile holds both h2 halves (each half
                    # is its own PSUM bank: accumulation groups stay clean)
                    sps = psS.tile([P, 2 * TQ], F32, tag="s", name="ps_s")
                    es = esb.tile([P, 2 * TQ], BF16, tag="es", name="es")
                    for h2 in range(2):
                        h = hp * 2 + h2
                        nc.tensor.matmul(
                            sps[:, h2 * TQ + jj:(h2 + 1) * TQ],
                            kT_sb[h][:, kb * P:(kb + 1) * P],
                            qT_sb[hp][:, qc * TQ + jj:(qc + 1) * TQ],
                            start=True, stop=True)
                    if not diag:  # jj == 0: one contiguous exp
                        nc.scalar.activation(es, sps, AF.Exp)
                    else:  # one strided exp over both live ranges
                        ev = es.rearrange("p (b c) -> p b c", c=TQ)[:, :, jj:TQ]
                        sv = sps.rearrange("p (b c) -> p b c", c=TQ)[:, :, jj:TQ]
                        nc.scalar.activation(ev, sv, AF.Exp)
                        for h2 in range(2):  # mask diagonal 128x128 squares
                            sq = es[:, h2 * TQ + jj:h2 * TQ + jj + P]
                            nc.vector.tensor_tensor(sq, sq, tri_sb,
                                                    op=ALU.mult)
                    if len(pending) == 5:
                        emit_pv(*pending.pop(0), norm=False)
                    pending.append((kb, jj, es))
                    blk_i += 1
                    # keep ~2 fillers in reserve for the flush region
                    while f_i < len(fillers) * blk_i // (nblk + 3):
                        fillers[f_i]()
                        f_i += 1
                for pi, item in enumerate(pending):
                    emit_pv(*item, norm=(pi == len(pending) - 1))
                    if f_i < len(fillers) and pi == 0:
                        fillers[f_i]()
                        f_i += 1
            while f_i < len(fillers):
                fillers[f_i]()
                f_i += 1

        _ofinal_i = [0]

        def oproj_chain(tb, jc, final):
            if final:
                # attention PSUM is dead by now: rotate over psh(2) + y0 +
                # y1 for 4-deep pipelining so evictions never pace the PE
                k = _ofinal_i[0] = _ofinal_i[0] + 1
                if k % 2 == 0:
                    ops = psh.tile([P, TQ], F32, tag="sh", name="ps_o")
                else:
                    ops = psY.tile([P, TQ], F32, tag=f"y{(k // 2) % 2}",
                                   name="ps_o")
            else:
                ops = psh.tile([P, TQ], F32, tag="sh", name="ps_o")
            for fb2 in range(2):
                nc.tensor.matmul(
                    ops, yT_sb[fb2][:, tb * P:(tb + 1) * P],
                    wo_sb[fb2][:, jc * TQ:(jc + 1) * TQ],
                    start=(fb2 == 0), stop=(fb2 == 1))
            # GPSIMD can't read PSUM. During attention ScalarE is the exp
            # pacer, so interleaved evictions stay on DVE; the final stage
            # (exp done) splits into two independent tiles so DVE+ScalarE
            # halves genuinely run in parallel, with one DMA each.
            r0 = (tb * 2 + jc) * P
            if final:
                # split across two engines + two DMA queues: PE-paced tail
                obA = osb.tile([P, TQ // 2], BF16, tag="obA", name="obA")
                obB = osb.tile([P, TQ // 2], BF16, tag="obB", name="obB")
                nc.vector.tensor_copy(obA, ops[:, 0:TQ // 2])
                nc.scalar.copy(obB, ops[:, TQ // 2:TQ])
                nc.scalar.dma_start(out_ap[r0:r0 + P, 0:TQ // 2], obA)
                nc.sync.dma_start(out_ap[r0:r0 + P, TQ // 2:TQ], obB)
            else:
                ob = osb.tile([P, TQ], BF16, tag="ob", name="ob")
                nc.vector.tensor_copy(ob, ops)
                nc.sync.dma_start(out_ap[r0:r0 + P, :], ob)

        def oproj_fillers(qc, final=False):
            return [(lambda tb=tb, jc=jc: oproj_chain(tb, jc, final))
                    for tb in range(qc * 4, (qc + 1) * 4)
                    for jc in range(CO // TQ)]

        # PE warmup: dummy matmuls on the (tiny, early-arriving) tri tile
        # during input DMA so the HAM clock-gate is released when real
        # work starts.
        for w in range(24):
            wps = psh.tile([P, TQ], F32, tag="sh", name="ps_warm")
            nc.tensor.matmul(wps[:, 0:P], tri_sb, tri_sb, start=True,
                             stop=True)

        # Stage pipeline: proj(i+1) between attn(i) and attn(i+1) hides
        # attn(i)'s normalize latency; oproj(i) chains are interleaved as
        # PE filler between attn(i+1)'s pairs (attention is mildly
        # exp-paced, so spare PE slots are filled with o_proj work).
        emit_proj(0)
        emit_attn(0)
        for i in range(1, NQC):
            emit_proj(i)
            emit_attn(i, fillers=oproj_fillers(i - 1))
        for f in oproj_fillers(NQC - 1, final=True):
            f()


def build_program(cfg: Cfg, num_cores: int):
    nc = bacc.Bacc("TRN2", target_bir_lowering=False, debug=False,
                   num_devices=num_cores)
    ins = {
        # chunk-major: [NQC * C, TQ] so per-(cb, chunk) DMAs are contiguous
        "xT": nc.dram_tensor("xT", [cfg.NQC * cfg.C, cfg.TQ], BF16,
                             kind="ExternalInput").ap(),
        "wqk": nc.dram_tensor("wqk", [cfg.C, 2 * cfg.HL * cfg.D], BF16,
                              kind="ExternalInput").ap(),
        "wv": nc.dram_tensor("wv", [cfg.C, cfg.HL * cfg.D], BF16,
                             kind="ExternalInput").ap(),
        "wo": nc.dram_tensor("wo", [cfg.HL * cfg.D, cfg.CO], BF16,
                             kind="ExternalInput").ap(),
        "bscale": nc.dram_tensor("bscale", [P, cfg.NFB], F32,
                                 kind="ExternalInput").ap(),
        "bbias": nc.dram_tensor("bbias", [P, cfg.NFB], F32,
                                kind="ExternalInput").ap(),
        "tri": nc.dram_tensor("tri", [P, P], BF16,
                              kind="ExternalInput").ap(),
    }
    # (tb, jc)-chunk-major: [TCB * (CO/TQ) * P, TQ], contiguous per chunk
    out_ap = nc.dram_tensor("out", [cfg.TCB * (cfg.CO // cfg.TQ) * P, cfg.TQ],
                            BF16, kind="ExternalOutput").ap()
    with tile.TileContext(nc) as tc:
        with ExitStack() as ctx:
            emit_kernel(tc, cfg, ins, out_ap, ctx)
    nc.compile()
    return nc


def prep_core_inputs(x_b: np.ndarray, W_qkv: np.ndarray, b_qkv: np.ndarray,
                     W_o: np.ndarray, heads, cfg: Cfg) -> dict:
    """x_b: [T, C] fp32 for this core's batch; heads: HL global head ids."""
    C, D, HL = cfg.C, cfg.D, cfg.HL
    scale = 1.0 / np.sqrt(D)
    qcols = np.concatenate([np.arange(h * D, (h + 1) * D) for h in heads])
    kcols = C + qcols
    vcols = 2 * C + qcols
    wqk = np.ascontiguousarray(
        np.concatenate([W_qkv[:, qcols], W_qkv[:, kcols]], axis=1)
    ).astype(NPBF16)
    wv = np.ascontiguousarray(W_qkv[:, vcols]).astype(NPBF16)
    wo = np.ascontiguousarray(W_o[qcols, :]).astype(NPBF16)
    bq = b_qkv[qcols].astype(np.float32)
    bk = b_qkv[kcols].astype(np.float32)
    scale_vec = np.concatenate([np.full(HL * D, scale, np.float32),
                                np.ones(HL * D, np.float32)])
    bias_vec = np.concatenate([bq * scale, bk])
    bscale = np.ascontiguousarray(scale_vec.reshape(cfg.NFB, P).T)
    bbias = np.ascontiguousarray(bias_vec.reshape(cfg.NFB, P).T)
    xT_full = x_b.T.astype(NPBF16)  # [C, T]
    # chunk-major: stack the NQC column chunks vertically -> [NQC*C, TQ]
    xT = np.ascontiguousarray(
        np.concatenate([xT_full[:, tq * cfg.TQ:(tq + 1) * cfg.TQ]
                        for tq in range(cfg.NQC)], axis=0))
    # tri[ki, qq] = 1 where qq >= ki (keep), else 0 — diagonal-square mask
    tri = np.triu(np.ones((P, P), np.float32)).astype(NPBF16)
    return {"xT": xT, "wqk": wqk, "wv": wv, "wo": wo,
            "bscale": bscale, "bbias": bbias, "tri": tri}


_PROGRAM_CACHE = {}


def _get_program(cfg: Cfg, num_cores: int):
    key = (cfg, num_cores)
    if key not in _PROGRAM_CACHE:
        _PROGRAM_CACHE[key] = build_program(cfg, num_cores)
    return _PROGRAM_CACHE[key]


LAST_RESULTS = None


def kernel(x: np.ndarray, W_qkv: np.ndarray, b_qkv: np.ndarray,
           W_o: np.ndarray, b_o: np.ndarray) -> np.ndarray:
    global LAST_RESULTS
    from concourse.bass_utils import run_bass_kernel_spmd

    x = np.asarray(x, np.float32)
    W_qkv = np.asarray(W_qkv, np.float32)
    b_qkv = np.asarray(b_qkv, np.float32)
    W_o = np.asarray(W_o, np.float32)
    b_o = np.asarray(b_o, np.float32)

    B, T, C = x.shape
    H = 16
    cfg = Cfg(T=T, C=C, CO=W_o.shape[1], D=C // H, HL=4)
    n_cores = 8
    groups = H // cfg.HL  # 4 head groups
    assert B * groups == n_cores

    nc = _get_program(cfg, n_cores)

    in_maps = []
    for core in range(n_cores):
        b, hg = core // groups, core % groups
        heads = list(range(hg * cfg.HL, (hg + 1) * cfg.HL))
        in_maps.append(prep_core_inputs(x[b], W_qkv, b_qkv, W_o, heads, cfg))

    res = run_bass_kernel_spmd(nc, in_maps, core_ids=list(range(n_cores)))
    LAST_RESULTS = res

    out = np.zeros((B, T, cfg.CO), np.float32)
    njc = cfg.CO // cfg.TQ
    for core in range(n_cores):
        raw = np.asarray(res.results[core]["out"], np.float32)
        part = raw.reshape(cfg.TCB, njc, P, cfg.TQ).transpose(0, 2, 1, 3)
        out[core // groups] += part.reshape(T, cfg.CO)
    # softmax rows sum to 1, so the v-bias contributes b_v @ W_o to every
    # output row; fold it into the output bias on the host.
    bias_full = b_o + b_qkv[2 * C:3 * C] @ W_o
    out += bias_full[None, None, :].astype(np.float32)
    return out


# revision 61
# speedup vs baseline: 1.0045x; 1.0045x over previous
"""Causal multi-head attention forward (B=2, T=2048, C=1024, H=16, D=64)
for 8 Trainium2 NeuronCores.

Sharding: core = (batch b, head-group hg) with b in {0,1}, hg in {0..3};
each core computes QKV projection for its 4 heads on its batch, causal
flash attention for those heads, and a partial output projection
(contraction over its 256 head-feature rows of W_o). Host sums the 4
partials per batch (f32) and adds b_o (+ folded b_v @ W_o).

Key layout decisions (per core):
  - All matmuls run with K=128 (PE stays in one tile config; switching
    between 64-row and 128-row configs costs ~130 ns per switch on HW).
    S = k^T q uses per-head stationary tiles kT[h] [128, T] where the
    64 rows matching the head's position in the packed q tile hold k
    and the other 64 rows are zeroed once at startup.
  - v_all groups [ones(64) | v(64)] per (t-block, head): the PV matmul
    emits the softmax denominator l duplicated on PSUM partitions 0..63
    and y^T on 64..127, so normalize is recip([64,TQ]) + one multiply
    (no partition broadcast).
  - Stages are interleaved: proj(0), attn(0), proj(1), oproj(0),
    attn(1), proj(2), oproj(1), attn(2), proj(3), oproj(2), attn(3),
    oproj(3). ScalarE exp overlaps projection/o_proj PE work, and the
    proj between attn(i) and oproj(i) hides the normalize latency.
  - Diagonal S blocks keep dead-column prefixes in persistent es tiles
    zeroed once at startup (exp writes live columns only); the diagonal
    128x128 square is masked by a DVE multiply with a host-provided
    triangular mask.
  - o_proj partials leave the chip as bf16 (host accumulates in f32),
    halving output DMA bytes.
"""

import os
import sys
from contextlib import ExitStack
from dataclasses import dataclass

import numpy as np

for _p in ("/opt/trn_rl_repo",):
    if _p not in sys.path and os.path.isdir(_p):
        sys.path.insert(0, _p)

import ml_dtypes

import concourse.bass as bass
import concourse.bacc as bacc
import concourse.mybir as mybir
import concourse.tile as tile


def _install_axon_ntff_hook():
    """Provide antenv.axon_hooks (absent on this image) so bass_utils'
    trace path works; registers the ctypes NTFF hook when available."""
    import types

    if "antenv.axon_hooks" not in sys.modules:
        import antenv

        mod = types.ModuleType("antenv.axon_hooks")
        _reg = [None]
        mod.get_axon_ntff_profile_hook = lambda: _reg[0]
        mod.set_axon_ntff_profile_hook = lambda h: _reg.__setitem__(0, h)
        sys.modules["antenv.axon_hooks"] = mod
        antenv.axon_hooks = mod
    hooks = sys.modules["antenv.axon_hooks"]
    if hooks.get_axon_ntff_profile_hook() is not None:
        return
    try:
        import contextlib
        import ctypes

        lib = ctypes.CDLL("/opt/axon/libaxon_pjrt.so")
        if not hasattr(lib, "axon_start_nrt_profile"):
            return
        lib.axon_start_nrt_profile.argtypes = [
            ctypes.POINTER(ctypes.c_int64), ctypes.c_size_t]
        lib.axon_start_nrt_profile.restype = ctypes.c_int64
        lib.axon_stop_nrt_profile.argtypes = [ctypes.c_char_p]
        lib.axon_stop_nrt_profile.restype = ctypes.c_int64

        @contextlib.contextmanager
        def _hook(output_dir, device_ids):
            import jax

            jax.devices()
            if device_ids:
                ids = (ctypes.c_int64 * len(device_ids))(*device_ids)
                rc = lib.axon_start_nrt_profile(ids, len(device_ids))
            else:
                rc = lib.axon_start_nrt_profile(None, 0)
            if rc != 0:
                raise RuntimeError(f"axon_start_nrt_profile rc={rc}")
            try:
                yield
            finally:
                n = lib.axon_stop_nrt_profile(str(output_dir).encode())
                print(f"ntff profile: {n} file(s) -> {output_dir}",
                      file=sys.stderr)

        hooks.set_axon_ntff_profile_hook(_hook)
    except Exception:
        pass


try:
    _install_axon_ntff_hook()
except Exception:
    pass

BF16 = mybir.dt.bfloat16
F32 = mybir.dt.float32
AF = mybir.ActivationFunctionType
ALU = mybir.AluOpType
NPBF16 = ml_dtypes.bfloat16

P = 128


@dataclass(frozen=True)
class Cfg:
    T: int = 2048  # sequence length
    C: int = 1024  # input feature dim
    CO: int = 1024  # output feature dim (W_o cols)
    D: int = 64  # head dim
    HL: int = 4  # local heads per core (2 row-packed pairs)
    TQ: int = 512  # query-chunk size

    @property
    def CB(self):  # c blocks
        return self.C // P

    @property
    def NFB(self):  # qk f-blocks (q+k for HL heads)
        return 2 * self.HL * self.D // P

    @property
    def NQC(self):  # query chunks
        return self.T // self.TQ

    @property
    def TCB(self):  # t blocks of 128 (ki blocks / o_proj rows)
        return self.T // P

    @property
    def VG(self):  # v group width: [ones(64) | v(64)]
        return self.D + 64


def emit_kernel(tc: tile.TileContext, cfg: Cfg, ins: dict, out_ap: bass.AP,
                ctx: ExitStack):
    nc = tc.nc
    T, C, CO, D, HL, TQ = cfg.T, cfg.C, cfg.CO, cfg.D, cfg.HL, cfg.TQ
    VG = cfg.VG
    NQC, CB = cfg.NQC, cfg.CB
    assert HL == 4 and D == 64 and TQ == 512

    io = ctx.enter_context(tc.tile_pool(name="io", bufs=1))

    # ---- persistent SBUF tiles + input DMA (priority order) ----
    tri_sb = io.tile([P, P], BF16, name="tri", tag="tri")
    nc.sync.dma_start(tri_sb, ins["tri"][:, :])  # first: warmup reads it
    bscale_sb = io.tile([P, cfg.NFB], F32, name="bscale", tag="bscale")
    nc.sync.dma_start(bscale_sb, ins["bscale"][:, :])
    bbias_sb = io.tile([P, cfg.NFB], F32, name="bbias", tag="bbias")
    nc.sync.dma_start(bbias_sb, ins["bbias"][:, :])
    # x^T in per-(cb, chunk) tiles; DRAM layout is chunk-major so every
    # chunk DMA is one fully-contiguous 128KB block
    xTc = [[io.tile([P, TQ], BF16, name=f"xT{cb}_{tq}", tag=f"xT{cb}_{tq}")
            for tq in range(NQC)] for cb in range(CB)]

    def dma_x_chunk(tq):
        for cb in range(CB):
            r0 = (tq * CB + cb) * P
            nc.sync.dma_start(xTc[cb][tq], ins["xT"][r0:r0 + P, :])

    # weights ride the second DMA queue (Activation HWDGE) in parallel
    # with x chunks on the SP queue — halves the start-up priming time.
    # ScalarE is idle until ~15us, so the queue occupancy is free.
    wqk_sb = []
    for cb in range(CB):
        wq = io.tile([P, 2 * HL * D], BF16, name=f"wqk{cb}", tag=f"wqk{cb}")
        nc.scalar.dma_start(wq, ins["wqk"][cb * P:(cb + 1) * P, :])
        wqk_sb.append(wq)
    dma_x_chunk(0)
    wv_sb = []
    for cb in range(CB):
        wvt = io.tile([P, HL * D], BF16, name=f"wv{cb}", tag=f"wv{cb}")
        nc.scalar.dma_start(wvt, ins["wv"][cb * P:(cb + 1) * P, :])
        wv_sb.append(wvt)
    dma_x_chunk(1)
    wo_sb = []
    for fb in range(HL * D // P):
        wot = io.tile([P, CO], BF16, name=f"wo{fb}", tag=f"wo{fb}")
        nc.sync.dma_start(wot, ins["wo"][fb * P:(fb + 1) * P, :])
        wo_sb.append(wot)
    dma_x_chunk(2)
    dma_x_chunk(3)

    # ---- persistent compute tiles ----
    qT_sb = [io.tile([P, T], BF16, name=f"qT{hp}", tag=f"qT{hp}")
             for hp in range(2)]
    # per-head k, zero-padded in the 64 rows not matching the head's slot
    # in the packed q tile (h even -> k at rows 0:64, h odd -> rows 64:128)
    kT_sb = [io.tile([P, T], BF16, name=f"kT{h}", tag=f"kT{h}")
             for h in range(HL)]
    v_all = io.tile([P, cfg.TCB * HL * VG], BF16, name="v_all", tag="v_all")
    yT_sb = [io.tile([P, T], BF16, name=f"yT{hp}", tag=f"yT{hp}")
             for hp in range(2)]
    rT = [io.tile([D, TQ], F32, name=f"rT{h2}", tag=f"rT{h2}")
          for h2 in range(2)]

    # ---- startup memsets, split across DVE and GpSimd ----
    nc.vector.memset(kT_sb[0][64:P, :], 0.0)
    nc.vector.memset(kT_sb[1][0:64, :], 0.0)
    nc.gpsimd.memset(kT_sb[2][64:P, :], 0.0)
    nc.gpsimd.memset(kT_sb[3][0:64, :], 0.0)
    # ones block of every v group (-> l duplicated on PSUM rows 0..63)
    ones_view = v_all.rearrange("p (g c) -> p g c", c=VG)[:, :, 0:D]
    nc.gpsimd.memset(ones_view, 1.0)

    with (tc.tile_pool(name="psh", bufs=2, space="PSUM") as psh,
          tc.tile_pool(name="psS", bufs=2, space="PSUM") as psS,
          tc.tile_pool(name="psY", bufs=1, space="PSUM") as psY,
          tc.tile_pool(name="esb", bufs=6) as esb,
          tc.tile_pool(name="osb", bufs=3) as osb):

        def emit_proj(tq):
            # q/k projection chains for this column chunk. Chains 2 and 3
            # borrow the (idle-between-attentions) psY banks so the first
            # chains never wait on the previous stage's eviction backlog.
            for fb in range(cfg.NFB):
                if tq > 0 and fb in (2, 3):
                    ps = psY.tile([P, TQ], F32, tag=f"y{fb - 2}",
                                  name="ps_qk")
                else:
                    ps = psh.tile([P, TQ], F32, tag="sh", name="ps_qk")
                for cb in range(CB):
                    nc.tensor.matmul(
                        ps, wqk_sb[cb][:, fb * P:(fb + 1) * P], xTc[cb][tq],
                        start=(cb == 0), stop=(cb == CB - 1))
                # evictions on DVE (tensor_scalar affine) — ScalarE is the
                # exp pacer and must not carry eviction work
                if fb < 2:  # q, packed 2 heads
                    nc.vector.tensor_scalar(
                        qT_sb[fb][:, tq * TQ:(tq + 1) * TQ], ps,
                        bscale_sb[:, fb:fb + 1], bbias_sb[:, fb:fb + 1],
                        op0=ALU.mult, op1=ALU.add)
                else:  # k, split per head into zero-padded tiles
                    for half in range(2):
                        h = (fb - 2) * 2 + half
                        r0, r1 = half * D, (half + 1) * D
                        nc.vector.tensor_scalar(
                            kT_sb[h][r0:r1, tq * TQ:(tq + 1) * TQ],
                            ps[r0:r1, :],
                            bscale_sb[r0:r1, fb:fb + 1],
                            bbias_sb[r0:r1, fb:fb + 1],
                            op0=ALU.mult, op1=ALU.add)
            # v projection: two t-blocks share one PSUM tile, evict on DVE
            for t2 in range(2):
                psv = psh.tile([P, TQ], F32, tag="sh", name="ps_v")
                for j in range(2):
                    for cb in range(CB):
                        nc.tensor.matmul(
                            psv[:, j * HL * D:(j + 1) * HL * D],
                            xTc[cb][tq][:, (t2 * 2 + j) * P:(t2 * 2 + j + 1) * P],
                            wv_sb[cb],
                            start=(cb == 0), stop=(cb == CB - 1))
                for j in range(2):
                    tb = tq * 4 + t2 * 2 + j
                    vdst = v_all[:, tb * HL * VG:(tb + 1) * HL * VG]
                    vdst = vdst.rearrange("p (h g) -> p h g", g=VG)[:, :, D:VG]
                    nc.vector.tensor_copy(
                        vdst,
                        psv[:, j * HL * D:(j + 1) * HL * D].rearrange(
                            "p (h d) -> p h d", d=D))

        def emit_attn(qc, fillers=()):
            # Block-granularity software pipeline: S(kb) overlaps exp(kb-1)
            # and PV(kb-1) (sps double-buffered per h2, es pool depth 3).
            # Diagonal blocks compute/exp/accumulate live columns only.
            # fillers: closures each emitting one o_proj chain of the
            # previous chunk, spread out to absorb the exp-latency slack.
            fillers = list(fillers)
            nkb = (qc + 1) * TQ // P
            f_i = 0
            blk_i = 0
            nblk = 2 * nkb
            for hp in range(2):
                yps = [psY.tile([P, TQ], F32, tag=f"y{h2}", name=f"ps_y{h2}")
                       for h2 in range(2)]

                def emit_pv(kb, jj, es, norm):
                    for h2 in range(2):
                        h = hp * 2 + h2
                        nc.tensor.matmul(
                            yps[h2][:, jj:TQ],
                            v_all[:, (kb * HL + h) * VG:(kb * HL + h + 1) * VG],
                            es[:, h2 * TQ + jj:(h2 + 1) * TQ],
                            start=(kb == 0), stop=(kb == nkb - 1),
                            skip_group_check=True)
                        if norm:  # l duplicated on PSUM rows 0..63
                            nc.vector.reciprocal_approx_fast(
                                rT[h2], yps[h2][0:D, :])
                            nc.vector.tensor_tensor(
                                yT_sb[hp][h2 * D:(h2 + 1) * D,
                                          qc * TQ:(qc + 1) * TQ],
                                yps[h2][D:2 * D, :], rT[h2], op=ALU.mult)

                # 2-block PV lag: exp(kb) (one merged instruction) has a
                # full extra PE block of slack before PV(kb) needs it, so
                # the PE never waits on ScalarE in steady state.
                pending = []
                for kb in range(nkb):
                    jj = max(0, kb * P - qc * TQ)
                    diag = kb * P >= qc * TQ
                    # one [128, 2*TQ] tile holds both h2 halves (each half
                    # is its own PSUM bank: accumulation groups stay clean)
                    sps = psS.tile([P, 2 * TQ], F32, tag="s", name="ps_s")
                    es = esb.tile([P, 2 * TQ], BF16, tag="es", name="es")
                    for h2 in range(2):
                        h = hp * 2 + h2
                        nc.tensor.matmul(
                            sps[:, h2 * TQ + jj:(h2 + 1) * TQ],
                            kT_sb[h][:, kb * P:(kb + 1) * P],
                            qT_sb[hp][:, qc * TQ + jj:(qc + 1) * TQ],
                            start=True, stop=True)
                    if not diag:  # jj == 0: one contiguous exp
                        nc.scalar.activation(es, sps, AF.Exp)
                    else:  # one strided exp over both live ranges
                        ev = es.rearrange("p (b c) -> p b c", c=TQ)[:, :, jj:TQ]
                        sv = sps.rearrange("p (b c) -> p b c", c=TQ)[:, :, jj:TQ]
                        nc.scalar.activation(ev, sv, AF.Exp)
                        for h2 in range(2):  # mask diagonal 128x128 squares
                            sq = es[:, h2 * TQ + jj:h2 * TQ + jj + P]
                            nc.vector.tensor_tensor(sq, sq, tri_sb,
                                                    op=ALU.mult)
                    if len(pending) == 5:
                        emit_pv(*pending.pop(0), norm=False)
                    pending.append((kb, jj, es))
                    blk_i += 1
                    # keep ~2 fillers in reserve for the flush region
                    while f_i < len(fillers) * blk_i // (nblk + 3):
                        fillers[f_i]()
                        f_i += 1
                for pi, item in enumerate(pending):
                    emit_pv(*item, norm=(pi == len(pending) - 1))
                    if f_i < len(fillers) and pi == 0:
                        fillers[f_i]()
                        f_i += 1
            while f_i < len(fillers):
                fillers[f_i]()
                f_i += 1

        _ofinal_i = [0]

        def oproj_chain(tb, jc, final):
            if final:
                # attention PSUM is dead by now: rotate over psh(2) + y0 +
                # y1 for 4-deep pipelining so evictions never pace the PE
                k = _ofinal_i[0] = _ofinal_i[0] + 1
                if k % 2 == 0:
                    ops = psh.tile([P, TQ], F32, tag="sh", name="ps_o")
                else:
                    ops = psY.tile([P, TQ], F32, tag=f"y{(k // 2) % 2}",
                                   name="ps_o")
            else:
                ops = psh.tile([P, TQ], F32, tag="sh", name="ps_o")
            for fb2 in range(2):
                nc.tensor.matmul(
                    ops, yT_sb[fb2][:, tb * P:(tb + 1) * P],
                    wo_sb[fb2][:, jc * TQ:(jc + 1) * TQ],
                    start=(fb2 == 0), stop=(fb2 == 1))
            # GPSIMD can't read PSUM. During attention ScalarE is the exp
            # pacer, so interleaved evictions stay on DVE; the final stage
            # (exp done) splits into two independent tiles so DVE+ScalarE
            # halves genuinely run in parallel, with one DMA each.
            r0 = (tb * 2 + jc) * P
            ob = osb.tile([P, TQ], BF16, tag="ob", name="ob")
            if final and (tb + jc) % 2 == 1:
                # final stage alternates eviction engines (exp is done, so
                # ScalarE is free); 4-deep PSUM keeps the tail PE-paced.
                # All DMAs stay on the idle SP queue — GpSimd DMAs would
                # cost a ~3.5us queue drain at kernel end.
                nc.scalar.copy(ob, ops)
            else:
                nc.vector.tensor_copy(ob, ops)
            nc.sync.dma_start(out_ap[r0:r0 + P, :], ob)

        def oproj_fillers(qc, final=False):
            return [(lambda tb=tb, jc=jc: oproj_chain(tb, jc, final))
                    for tb in range(qc * 4, (qc + 1) * 4)
                    for jc in range(CO // TQ)]

        # PE warmup: dummy matmuls on the (tiny, early-arriving) tri tile
        # during input DMA so the HAM clock-gate is released when real
        # work starts.
        for w in range(24):
            wps = psh.tile([P, TQ], F32, tag="sh", name="ps_warm")
            nc.tensor.matmul(wps[:, 0:P], tri_sb, tri_sb, start=True,
                             stop=True)

        # Stage pipeline: proj(i+1) between attn(i) and attn(i+1) hides
        # attn(i)'s normalize latency; oproj(i) chains are interleaved as
        # PE filler between attn(i+1)'s pairs (attention is mildly
        # exp-paced, so spare PE slots are filled with o_proj work).
        emit_proj(0)
        emit_attn(0)
        for i in range(1, NQC):
            emit_proj(i)
            emit_attn(i, fillers=oproj_fillers(i - 1))
        for f in oproj_fillers(NQC - 1, final=True):
            f()


def build_program(cfg: Cfg, num_cores: int):
    nc = bacc.Bacc("TRN2", target_bir_lowering=False, debug=False,
                   num_devices=num_cores)
    ins = {
        # chunk-major: [NQC * C, TQ] so per-(cb, chunk) DMAs are contiguous
        "xT": nc.dram_tensor("xT", [cfg.NQC * cfg.C, cfg.TQ], BF16,
                             kind="ExternalInput").ap(),
        "wqk": nc.dram_tensor("wqk", [cfg.C, 2 * cfg.HL * cfg.D], BF16,
                              kind="ExternalInput").ap(),
        "wv": nc.dram_tensor("wv", [cfg.C, cfg.HL * cfg.D], BF16,
                             kind="ExternalInput").ap(),
        "wo": nc.dram_tensor("wo", [cfg.HL * cfg.D, cfg.CO], BF16,
                             kind="ExternalInput").ap(),
        "bscale": nc.dram_tensor("bscale", [P, cfg.NFB], F32,
                                 kind="ExternalInput").ap(),
        "bbias": nc.dram_tensor("bbias", [P, cfg.NFB], F32,
                                kind="ExternalInput").ap(),
        "tri": nc.dram_tensor("tri", [P, P], BF16,
                              kind="ExternalInput").ap(),
    }
    # (tb, jc)-chunk-major: [TCB * (CO/TQ) * P, TQ], contiguous per chunk
    out_ap = nc.dram_tensor("out", [cfg.TCB * (cfg.CO // cfg.TQ) * P, cfg.TQ],
                            BF16, kind="ExternalOutput").ap()
    with tile.TileContext(nc) as tc:
        with ExitStack() as ctx:
            emit_kernel(tc, cfg, ins, out_ap, ctx)
    nc.compile()
    return nc


def prep_core_inputs(x_b: np.ndarray, W_qkv: np.ndarray, b_qkv: np.ndarray,
                     W_o: np.ndarray, heads, cfg: Cfg) -> dict:
    """x_b: [T, C] fp32 for this core's batch; heads: HL global head ids."""
    C, D, HL = cfg.C, cfg.D, cfg.HL
    scale = 1.0 / np.sqrt(D)
    qcols = np.concatenate([np.arange(h * D, (h + 1) * D) for h in heads])
    kcols = C + qcols
    vcols = 2 * C + qcols
    wqk = np.ascontiguousarray(
        np.concatenate([W_qkv[:, qcols], W_qkv[:, kcols]], axis=1)
    ).astype(NPBF16)
    wv = np.ascontiguousarray(W_qkv[:, vcols]).astype(NPBF16)
    wo = np.ascontiguousarray(W_o[qcols, :]).astype(NPBF16)
    bq = b_qkv[qcols].astype(np.float32)
    bk = b_qkv[kcols].astype(np.float32)
    scale_vec = np.concatenate([np.full(HL * D, scale, np.float32),
                                np.ones(HL * D, np.float32)])
    bias_vec = np.concatenate([bq * scale, bk])
    bscale = np.ascontiguousarray(scale_vec.reshape(cfg.NFB, P).T)
    bbias = np.ascontiguousarray(bias_vec.reshape(cfg.NFB, P).T)
    xT_full = x_b.T.astype(NPBF16)  # [C, T]
    # chunk-major: stack the NQC column chunks vertically -> [NQC*C, TQ]
    xT = np.ascontiguousarray(
        np.concatenate([xT_full[:, tq * cfg.TQ:(tq + 1) * cfg.TQ]
                        for tq in range(cfg.NQC)], axis=0))
    # tri[ki, qq] = 1 where qq >= ki (keep), else 0 — diagonal-square mask
    tri = np.triu(np.ones((P, P), np.float32)).astype(NPBF16)
    return {"xT": xT, "wqk": wqk, "wv": wv, "wo": wo,
            "bscale": bscale, "bbias": bbias, "tri": tri}


_PROGRAM_CACHE = {}


def _get_program(cfg: Cfg, num_cores: int):
    key = (cfg, num_cores)
    if key not in _PROGRAM_CACHE:
        _PROGRAM_CACHE[key] = build_program(cfg, num_cores)
    return _PROGRAM_CACHE[key]


LAST_RESULTS = None


def kernel(x: np.ndarray, W_qkv: np.ndarray, b_qkv: np.ndarray,
           W_o: np.ndarray, b_o: np.ndarray) -> np.ndarray:
    global LAST_RESULTS
    from concourse.bass_utils import run_bass_kernel_spmd

    x = np.asarray(x, np.float32)
    W_qkv = np.asarray(W_qkv, np.float32)
    b_qkv = np.asarray(b_qkv, np.float32)
    W_o = np.asarray(W_o, np.float32)
    b_o = np.asarray(b_o, np.float32)

    B, T, C = x.shape
    H = 16
    cfg = Cfg(T=T, C=C, CO=W_o.shape[1], D=C // H, HL=4)
    n_cores = 8
    groups = H // cfg.HL  # 4 head groups
    assert B * groups == n_cores

    nc = _get_program(cfg, n_cores)

    in_maps = []
    for core in range(n_cores):
        b, hg = core // groups, core % groups
        heads = list(range(hg * cfg.HL, (hg + 1) * cfg.HL))
        in_maps.append(prep_core_inputs(x[b], W_qkv, b_qkv, W_o, heads, cfg))

    res = run_bass_kernel_spmd(nc, in_maps, core_ids=list(range(n_cores)))
    LAST_RESULTS = res

    out = np.zeros((B, T, cfg.CO), np.float32)
    njc = cfg.CO // cfg.TQ
    for core in range(n_cores):
        raw = np.asarray(res.results[core]["out"], np.float32)
        part = raw.reshape(cfg.TCB, njc, P, cfg.TQ).transpose(0, 2, 1, 3)
        out[core // groups] += part.reshape(T, cfg.CO)
    # softmax rows sum to 1, so the v-bias contributes b_v @ W_o to every
    # output row; fold it into the output bias on the host.
    bias_full = b_o + b_qkv[2 * C:3 * C] @ W_o
    out += bias_full[None, None, :].astype(np.float32)
    return out


# revision 62
# speedup vs baseline: 1.0178x; 1.0133x over previous
"""Causal multi-head attention forward (B=2, T=2048, C=1024, H=16, D=64)
for 8 Trainium2 NeuronCores.

Sharding: core = (batch b, head-group hg) with b in {0,1}, hg in {0..3};
each core computes QKV projection for its 4 heads on its batch, causal
flash attention for those heads, and a partial output projection
(contraction over its 256 head-feature rows of W_o). Host sums the 4
partials per batch (f32) and adds b_o (+ folded b_v @ W_o).

Key layout decisions (per core):
  - All matmuls run with K=128 (PE stays in one tile config; switching
    between 64-row and 128-row configs costs ~130 ns per switch on HW).
    S = k^T q uses per-head stationary tiles kT[h] [128, T] where the
    64 rows matching the head's position in the packed q tile hold k
    and the other 64 rows are zeroed once at startup.
  - v_all groups [ones(64) | v(64)] per (t-block, head): the PV matmul
    emits the softmax denominator l duplicated on PSUM partitions 0..63
    and y^T on 64..127, so normalize is recip([64,TQ]) + one multiply
    (no partition broadcast).
  - Stages are interleaved: proj(0), attn(0), proj(1), oproj(0),
    attn(1), proj(2), oproj(1), attn(2), proj(3), oproj(2), attn(3),
    oproj(3). ScalarE exp overlaps projection/o_proj PE work, and the
    proj between attn(i) and oproj(i) hides the normalize latency.
  - Diagonal S blocks keep dead-column prefixes in persistent es tiles
    zeroed once at startup (exp writes live columns only); the diagonal
    128x128 square is masked by a DVE multiply with a host-provided
    triangular mask.
  - o_proj partials leave the chip as bf16 (host accumulates in f32),
    halving output DMA bytes.
"""

import os
import sys
from contextlib import ExitStack
from dataclasses import dataclass

import numpy as np

for _p in ("/opt/trn_rl_repo",):
    if _p not in sys.path and os.path.isdir(_p):
        sys.path.insert(0, _p)

import ml_dtypes

import concourse.bass as bass
import concourse.bacc as bacc
import concourse.mybir as mybir
import concourse.tile as tile


def _install_axon_ntff_hook():
    """Provide antenv.axon_hooks (absent on this image) so bass_utils'
    trace path works; registers the ctypes NTFF hook when available."""
    import types

    if "antenv.axon_hooks" not in sys.modules:
        import antenv

        mod = types.ModuleType("antenv.axon_hooks")
        _reg = [None]
        mod.get_axon_ntff_profile_hook = lambda: _reg[0]
        mod.set_axon_ntff_profile_hook = lambda h: _reg.__setitem__(0, h)
        sys.modules["antenv.axon_hooks"] = mod
        antenv.axon_hooks = mod
    hooks = sys.modules["antenv.axon_hooks"]
    if hooks.get_axon_ntff_profile_hook() is not None:
        return
    try:
        import contextlib
        import ctypes

        lib = ctypes.CDLL("/opt/axon/libaxon_pjrt.so")
        if not hasattr(lib, "axon_start_nrt_profile"):
            return
        lib.axon_start_nrt_profile.argtypes = [
            ctypes.POINTER(ctypes.c_int64), ctypes.c_size_t]
        lib.axon_start_nrt_profile.restype = ctypes.c_int64
        lib.axon_stop_nrt_profile.argtypes = [ctypes.c_char_p]
        lib.axon_stop_nrt_profile.restype = ctypes.c_int64

        @contextlib.contextmanager
        def _hook(output_dir, device_ids):
            import jax

            jax.devices()
            if device_ids:
                ids = (ctypes.c_int64 * len(device_ids))(*device_ids)
                rc = lib.axon_start_nrt_profile(ids, len(device_ids))
            else:
                rc = lib.axon_start_nrt_profile(None, 0)
            if rc != 0:
                raise RuntimeError(f"axon_start_nrt_profile rc={rc}")
            try:
                yield
            finally:
                n = lib.axon_stop_nrt_profile(str(output_dir).encode())
                print(f"ntff profile: {n} file(s) -> {output_dir}",
                      file=sys.stderr)

        hooks.set_axon_ntff_profile_hook(_hook)
    except Exception:
        pass


try:
    _install_axon_ntff_hook()
except Exception:
    pass

BF16 = mybir.dt.bfloat16
F32 = mybir.dt.float32
AF = mybir.ActivationFunctionType
ALU = mybir.AluOpType
NPBF16 = ml_dtypes.bfloat16

P = 128


@dataclass(frozen=True)
class Cfg:
    T: int = 2048  # sequence length
    C: int = 1024  # input feature dim
    CO: int = 1024  # output feature dim (W_o cols)
    D: int = 64  # head dim
    HL: int = 4  # local heads per core (2 row-packed pairs)
    TQ: int = 512  # query-chunk size

    @property
    def CB(self):  # c blocks
        return self.C // P

    @property
    def NFB(self):  # qk f-blocks (q+k for HL heads)
        return 2 * self.HL * self.D // P

    @property
    def NQC(self):  # query chunks
        return self.T // self.TQ

    @property
    def TCB(self):  # t blocks of 128 (ki blocks / o_proj rows)
        return self.T // P

    @property
    def VG(self):  # v group width: [ones(64) | v(64)]
        return self.D + 64


def emit_kernel(tc: tile.TileContext, cfg: Cfg, ins: dict, out_ap: bass.AP,
                ctx: ExitStack):
    nc = tc.nc
    T, C, CO, D, HL, TQ = cfg.T, cfg.C, cfg.CO, cfg.D, cfg.HL, cfg.TQ
    VG = cfg.VG
    NQC, CB = cfg.NQC, cfg.CB
    assert HL == 4 and D == 64 and TQ == 512

    io = ctx.enter_context(tc.tile_pool(name="io", bufs=1))

    # ---- persistent SBUF tiles + input DMA (priority order) ----
    tri_sb = io.tile([P, P], BF16, name="tri", tag="tri")
    nc.sync.dma_start(tri_sb, ins["tri"][:, :])  # first: warmup reads it
    bscale_sb = io.tile([P, cfg.NFB], F32, name="bscale", tag="bscale")
    nc.sync.dma_start(bscale_sb, ins["bscale"][:, :])
    bbias_sb = io.tile([P, cfg.NFB], F32, name="bbias", tag="bbias")
    nc.sync.dma_start(bbias_sb, ins["bbias"][:, :])
    # x^T in per-(cb, chunk) tiles; DRAM layout is chunk-major so every
    # chunk DMA is one fully-contiguous 128KB block
    xTc = [[io.tile([P, TQ], BF16, name=f"xT{cb}_{tq}", tag=f"xT{cb}_{tq}")
            for tq in range(NQC)] for cb in range(CB)]

    def dma_x_chunk(tq):
        for cb in range(CB):
            r0 = (tq * CB + cb) * P
            nc.sync.dma_start(xTc[cb][tq], ins["xT"][r0:r0 + P, :])

    # weights ride the second DMA queue (Activation HWDGE) in parallel
    # with x chunks on the SP queue — halves the start-up priming time.
    # ScalarE is idle until ~15us, so the queue occupancy is free.
    wqk_sb = []
    for cb in range(CB):
        wq = io.tile([P, 2 * HL * D], BF16, name=f"wqk{cb}", tag=f"wqk{cb}")
        nc.scalar.dma_start(wq, ins["wqk"][cb * P:(cb + 1) * P, :])
        wqk_sb.append(wq)
    dma_x_chunk(0)
    wv_sb = []
    for cb in range(CB):
        wvt = io.tile([P, HL * D], BF16, name=f"wv{cb}", tag=f"wv{cb}")
        nc.scalar.dma_start(wvt, ins["wv"][cb * P:(cb + 1) * P, :])
        wv_sb.append(wvt)
    dma_x_chunk(1)
    wo_sb = []
    for fb in range(HL * D // P):
        wot = io.tile([P, CO], BF16, name=f"wo{fb}", tag=f"wo{fb}")
        nc.sync.dma_start(wot, ins["wo"][fb * P:(fb + 1) * P, :])
        wo_sb.append(wot)
    dma_x_chunk(2)
    dma_x_chunk(3)

    # ---- persistent compute tiles ----
    qT_sb = [io.tile([P, T], BF16, name=f"qT{hp}", tag=f"qT{hp}")
             for hp in range(2)]
    # per-head k, zero-padded in the 64 rows not matching the head's slot
    # in the packed q tile (h even -> k at rows 0:64, h odd -> rows 64:128)
    kT_sb = [io.tile([P, T], BF16, name=f"kT{h}", tag=f"kT{h}")
             for h in range(HL)]
    v_all = io.tile([P, cfg.TCB * HL * VG], BF16, name="v_all", tag="v_all")
    yT_sb = [io.tile([P, T], BF16, name=f"yT{hp}", tag=f"yT{hp}")
             for hp in range(2)]
    rT = [io.tile([D, TQ], F32, name=f"rT{h2}", tag=f"rT{h2}")
          for h2 in range(2)]
    warm_sb = io.tile([P, P], BF16, name="warm_sb", tag="warm_sb")

    # ---- startup memsets, split across DVE and GpSimd ----
    # warm tile first: memset completes ~2us before any DMA could land,
    # so PE warmup starts at engine-init time
    nc.gpsimd.memset(warm_sb, 0.0)
    nc.vector.memset(kT_sb[0][64:P, :], 0.0)
    nc.vector.memset(kT_sb[1][0:64, :], 0.0)
    nc.gpsimd.memset(kT_sb[2][64:P, :], 0.0)
    nc.gpsimd.memset(kT_sb[3][0:64, :], 0.0)
    # ones block of every v group (-> l duplicated on PSUM rows 0..63)
    ones_view = v_all.rearrange("p (g c) -> p g c", c=VG)[:, :, 0:D]
    nc.gpsimd.memset(ones_view, 1.0)

    with (tc.tile_pool(name="psh", bufs=2, space="PSUM") as psh,
          tc.tile_pool(name="psS", bufs=2, space="PSUM") as psS,
          tc.tile_pool(name="psY", bufs=1, space="PSUM") as psY,
          tc.tile_pool(name="esb", bufs=6) as esb,
          tc.tile_pool(name="osb", bufs=3) as osb):

        def emit_proj(tq):
            # q/k projection chains for this column chunk. Chains 2 and 3
            # borrow the (idle-between-attentions) psY banks so the first
            # chains never wait on the previous stage's eviction backlog.
            for fb in range(cfg.NFB):
                if tq > 0 and fb in (2, 3):
                    ps = psY.tile([P, TQ], F32, tag=f"y{fb - 2}",
                                  name="ps_qk")
                else:
                    ps = psh.tile([P, TQ], F32, tag="sh", name="ps_qk")
                for cb in range(CB):
                    nc.tensor.matmul(
                        ps, wqk_sb[cb][:, fb * P:(fb + 1) * P], xTc[cb][tq],
                        start=(cb == 0), stop=(cb == CB - 1))
                # evictions on DVE (tensor_scalar affine) — ScalarE is the
                # exp pacer and must not carry eviction work
                if fb < 2:  # q, packed 2 heads
                    nc.vector.tensor_scalar(
                        qT_sb[fb][:, tq * TQ:(tq + 1) * TQ], ps,
                        bscale_sb[:, fb:fb + 1], bbias_sb[:, fb:fb + 1],
                        op0=ALU.mult, op1=ALU.add)
                else:  # k, split per head into zero-padded tiles
                    for half in range(2):
                        h = (fb - 2) * 2 + half
                        r0, r1 = half * D, (half + 1) * D
                        nc.vector.tensor_scalar(
                            kT_sb[h][r0:r1, tq * TQ:(tq + 1) * TQ],
                            ps[r0:r1, :],
                            bscale_sb[r0:r1, fb:fb + 1],
                            bbias_sb[r0:r1, fb:fb + 1],
                            op0=ALU.mult, op1=ALU.add)
            # v projection: two t-blocks share one PSUM tile, evict on DVE
            for t2 in range(2):
                psv = psh.tile([P, TQ], F32, tag="sh", name="ps_v")
                for j in range(2):
                    for cb in range(CB):
                        nc.tensor.matmul(
                            psv[:, j * HL * D:(j + 1) * HL * D],
                            xTc[cb][tq][:, (t2 * 2 + j) * P:(t2 * 2 + j + 1) * P],
                            wv_sb[cb],
                            start=(cb == 0), stop=(cb == CB - 1))
                for j in range(2):
                    tb = tq * 4 + t2 * 2 + j
                    vdst = v_all[:, tb * HL * VG:(tb + 1) * HL * VG]
                    vdst = vdst.rearrange("p (h g) -> p h g", g=VG)[:, :, D:VG]
                    nc.vector.tensor_copy(
                        vdst,
                        psv[:, j * HL * D:(j + 1) * HL * D].rearrange(
                            "p (h d) -> p h d", d=D))

        def emit_attn(qc, fillers=()):
            # Block-granularity software pipeline: S(kb) overlaps exp(kb-1)
            # and PV(kb-1) (sps double-buffered per h2, es pool depth 3).
            # Diagonal blocks compute/exp/accumulate live columns only.
            # fillers: closures each emitting one o_proj chain of the
            # previous chunk, spread out to absorb the exp-latency slack.
            fillers = list(fillers)
            nkb = (qc + 1) * TQ // P
            f_i = 0
            blk_i = 0
            nblk = 2 * nkb
            for hp in range(2):
                yps = [psY.tile([P, TQ], F32, tag=f"y{h2}", name=f"ps_y{h2}")
                       for h2 in range(2)]

                def emit_pv(kb, jj, es, norm):
                    for h2 in range(2):
                        h = hp * 2 + h2
                        nc.tensor.matmul(
                            yps[h2][:, jj:TQ],
                            v_all[:, (kb * HL + h) * VG:(kb * HL + h + 1) * VG],
                            es[:, h2 * TQ + jj:(h2 + 1) * TQ],
                            start=(kb == 0), stop=(kb == nkb - 1),
                            skip_group_check=True)
                        if norm:  # l duplicated on PSUM rows 0..63
                            nc.vector.reciprocal_approx_fast(
                                rT[h2], yps[h2][0:D, :])
                            nc.vector.tensor_tensor(
                                yT_sb[hp][h2 * D:(h2 + 1) * D,
                                          qc * TQ:(qc + 1) * TQ],
                                yps[h2][D:2 * D, :], rT[h2], op=ALU.mult)

                # 2-block PV lag: exp(kb) (one merged instruction) has a
                # full extra PE block of slack before PV(kb) needs it, so
                # the PE never waits on ScalarE in steady state.
                pending = []
                for kb in range(nkb):
                    jj = max(0, kb * P - qc * TQ)
                    diag = kb * P >= qc * TQ
                    # one [128, 2*TQ] tile holds both h2 halves (each half
                    # is its own PSUM bank: accumulation groups stay clean)
                    sps = psS.tile([P, 2 * TQ], F32, tag="s", name="ps_s")
                    es = esb.tile([P, 2 * TQ], BF16, tag="es", name="es")
                    for h2 in range(2):
                        h = hp * 2 + h2
                        nc.tensor.matmul(
                            sps[:, h2 * TQ + jj:(h2 + 1) * TQ],
                            kT_sb[h][:, kb * P:(kb + 1) * P],
                            qT_sb[hp][:, qc * TQ + jj:(qc + 1) * TQ],
                            start=True, stop=True)
                    if not diag:  # jj == 0: one contiguous exp
                        nc.scalar.activation(es, sps, AF.Exp)
                    else:  # one strided exp over both live ranges
                        ev = es.rearrange("p (b c) -> p b c", c=TQ)[:, :, jj:TQ]
                        sv = sps.rearrange("p (b c) -> p b c", c=TQ)[:, :, jj:TQ]
                        nc.scalar.activation(ev, sv, AF.Exp)
                        for h2 in range(2):  # mask diagonal 128x128 squares
                            sq = es[:, h2 * TQ + jj:h2 * TQ + jj + P]
                            nc.vector.tensor_tensor(sq, sq, tri_sb,
                                                    op=ALU.mult)
                    if len(pending) == 5:
                        emit_pv(*pending.pop(0), norm=False)
                    pending.append((kb, jj, es))
                    blk_i += 1
                    # keep ~2 fillers in reserve for the flush region
                    while f_i < len(fillers) * blk_i // (nblk + 3):
                        fillers[f_i]()
                        f_i += 1
                for pi, item in enumerate(pending):
                    emit_pv(*item, norm=(pi == len(pending) - 1))
                    if f_i < len(fillers) and pi == 0:
                        fillers[f_i]()
                        f_i += 1
            while f_i < len(fillers):
                fillers[f_i]()
                f_i += 1

        _ofinal_i = [0]

        def oproj_chain(tb, jc, final):
            if final:
                # attention PSUM is dead by now: rotate over psh(2) + y0 +
                # y1 for 4-deep pipelining so evictions never pace the PE
                k = _ofinal_i[0] = _ofinal_i[0] + 1
                if k % 2 == 0:
                    ops = psh.tile([P, TQ], F32, tag="sh", name="ps_o")
                else:
                    ops = psY.tile([P, TQ], F32, tag=f"y{(k // 2) % 2}",
                                   name="ps_o")
            else:
                ops = psh.tile([P, TQ], F32, tag="sh", name="ps_o")
            for fb2 in range(2):
                nc.tensor.matmul(
                    ops, yT_sb[fb2][:, tb * P:(tb + 1) * P],
                    wo_sb[fb2][:, jc * TQ:(jc + 1) * TQ],
                    start=(fb2 == 0), stop=(fb2 == 1))
            # GPSIMD can't read PSUM. During attention ScalarE is the exp
            # pacer, so interleaved evictions stay on DVE; the final stage
            # (exp done) splits into two independent tiles so DVE+ScalarE
            # halves genuinely run in parallel, with one DMA each.
            r0 = (tb * 2 + jc) * P
            ob = osb.tile([P, TQ], BF16, tag="ob", name="ob")
            if final and (tb + jc) % 2 == 1:
                # final stage alternates eviction engines (exp is done, so
                # ScalarE is free); 4-deep PSUM keeps the tail PE-paced.
                # All DMAs stay on the idle SP queue — GpSimd DMAs would
                # cost a ~3.5us queue drain at kernel end.
                nc.scalar.copy(ob, ops)
            else:
                nc.vector.tensor_copy(ob, ops)
            nc.sync.dma_start(out_ap[r0:r0 + P, :], ob)

        def oproj_fillers(qc, final=False):
            return [(lambda tb=tb, jc=jc: oproj_chain(tb, jc, final))
                    for tb in range(qc * 4, (qc + 1) * 4)
                    for jc in range(CO // TQ)]

        # PE warmup: dummy matmuls on the (tiny, early-arriving) tri tile
        # during input DMA so the HAM clock-gate is released when real
        # work starts.
        for w in range(48):
            wps = psh.tile([P, TQ], F32, tag="sh", name="ps_warm")
            nc.tensor.matmul(wps[:, 0:P], warm_sb, warm_sb, start=True,
                             stop=True)

        # Stage pipeline: proj(i+1) between attn(i) and attn(i+1) hides
        # attn(i)'s normalize latency; oproj(i) chains are interleaved as
        # PE filler between attn(i+1)'s pairs (attention is mildly
        # exp-paced, so spare PE slots are filled with o_proj work).
        emit_proj(0)
        emit_attn(0)
        for i in range(1, NQC):
            emit_proj(i)
            emit_attn(i, fillers=oproj_fillers(i - 1))
        for f in oproj_fillers(NQC - 1, final=True):
            f()


def build_program(cfg: Cfg, num_cores: int):
    nc = bacc.Bacc("TRN2", target_bir_lowering=False, debug=False,
                   num_devices=num_cores)
    ins = {
        # chunk-major: [NQC * C, TQ] so per-(cb, chunk) DMAs are contiguous
        "xT": nc.dram_tensor("xT", [cfg.NQC * cfg.C, cfg.TQ], BF16,
                             kind="ExternalInput").ap(),
        "wqk": nc.dram_tensor("wqk", [cfg.C, 2 * cfg.HL * cfg.D], BF16,
                              kind="ExternalInput").ap(),
        "wv": nc.dram_tensor("wv", [cfg.C, cfg.HL * cfg.D], BF16,
                             kind="ExternalInput").ap(),
        "wo": nc.dram_tensor("wo", [cfg.HL * cfg.D, cfg.CO], BF16,
                             kind="ExternalInput").ap(),
        "bscale": nc.dram_tensor("bscale", [P, cfg.NFB], F32,
                                 kind="ExternalInput").ap(),
        "bbias": nc.dram_tensor("bbias", [P, cfg.NFB], F32,
                                kind="ExternalInput").ap(),
        "tri": nc.dram_tensor("tri", [P, P], BF16,
                              kind="ExternalInput").ap(),
    }
    # (tb, jc)-chunk-major: [TCB * (CO/TQ) * P, TQ], contiguous per chunk
    out_ap = nc.dram_tensor("out", [cfg.TCB * (cfg.CO // cfg.TQ) * P, cfg.TQ],
                            BF16, kind="ExternalOutput").ap()
    with tile.TileContext(nc) as tc:
        with ExitStack() as ctx:
            emit_kernel(tc, cfg, ins, out_ap, ctx)
    nc.compile()
    return nc


def prep_core_inputs(x_b: np.ndarray, W_qkv: np.ndarray, b_qkv: np.ndarray,
                     W_o: np.ndarray, heads, cfg: Cfg) -> dict:
    """x_b: [T, C] fp32 for this core's batch; heads: HL global head ids."""
    C, D, HL = cfg.C, cfg.D, cfg.HL
    scale = 1.0 / np.sqrt(D)
    qcols = np.concatenate([np.arange(h * D, (h + 1) * D) for h in heads])
    kcols = C + qcols
    vcols = 2 * C + qcols
    wqk = np.ascontiguousarray(
        np.concatenate([W_qkv[:, qcols], W_qkv[:, kcols]], axis=1)
    ).astype(NPBF16)
    wv = np.ascontiguousarray(W_qkv[:, vcols]).astype(NPBF16)
    wo = np.ascontiguousarray(W_o[qcols, :]).astype(NPBF16)
    bq = b_qkv[qcols].astype(np.float32)
    bk = b_qkv[kcols].astype(np.float32)
    scale_vec = np.concatenate([np.full(HL * D, scale, np.float32),
                                np.ones(HL * D, np.float32)])
    bias_vec = np.concatenate([bq * scale, bk])
    bscale = np.ascontiguousarray(scale_vec.reshape(cfg.NFB, P).T)
    bbias = np.ascontiguousarray(bias_vec.reshape(cfg.NFB, P).T)
    xT_full = x_b.T.astype(NPBF16)  # [C, T]
    # chunk-major: stack the NQC column chunks vertically -> [NQC*C, TQ]
    xT = np.ascontiguousarray(
        np.concatenate([xT_full[:, tq * cfg.TQ:(tq + 1) * cfg.TQ]
                        for tq in range(cfg.NQC)], axis=0))
    # tri[ki, qq] = 1 where qq >= ki (keep), else 0 — diagonal-square mask
    tri = np.triu(np.ones((P, P), np.float32)).astype(NPBF16)
    return {"xT": xT, "wqk": wqk, "wv": wv, "wo": wo,
            "bscale": bscale, "bbias": bbias, "tri": tri}


_PROGRAM_CACHE = {}


def _get_program(cfg: Cfg, num_cores: int):
    key = (cfg, num_cores)
    if key not in _PROGRAM_CACHE:
        _PROGRAM_CACHE[key] = build_program(cfg, num_cores)
    return _PROGRAM_CACHE[key]


LAST_RESULTS = None


def kernel(x: np.ndarray, W_qkv: np.ndarray, b_qkv: np.ndarray,
           W_o: np.ndarray, b_o: np.ndarray) -> np.ndarray:
    global LAST_RESULTS
    from concourse.bass_utils import run_bass_kernel_spmd

    x = np.asarray(x, np.float32)
    W_qkv = np.asarray(W_qkv, np.float32)
    b_qkv = np.asarray(b_qkv, np.float32)
    W_o = np.asarray(W_o, np.float32)
    b_o = np.asarray(b_o, np.float32)

    B, T, C = x.shape
    H = 16
    cfg = Cfg(T=T, C=C, CO=W_o.shape[1], D=C // H, HL=4)
    n_cores = 8
    groups = H // cfg.HL  # 4 head groups
    assert B * groups == n_cores

    nc = _get_program(cfg, n_cores)

    in_maps = []
    for core in range(n_cores):
        b, hg = core // groups, core % groups
        heads = list(range(hg * cfg.HL, (hg + 1) * cfg.HL))
        in_maps.append(prep_core_inputs(x[b], W_qkv, b_qkv, W_o, heads, cfg))

    res = run_bass_kernel_spmd(nc, in_maps, core_ids=list(range(n_cores)))
    LAST_RESULTS = res

    out = np.zeros((B, T, cfg.CO), np.float32)
    njc = cfg.CO // cfg.TQ
    for core in range(n_cores):
        raw = np.asarray(res.results[core]["out"], np.float32)
        part = raw.reshape(cfg.TCB, njc, P, cfg.TQ).transpose(0, 2, 1, 3)
        out[core // groups] += part.reshape(T, cfg.CO)
    # softmax rows sum to 1, so the v-bias contributes b_v @ W_o to every
    # output row; fold it into the output bias on the host.
    bias_full = b_o + b_qkv[2 * C:3 * C] @ W_o
    out += bias_full[None, None, :].astype(np.float32)
    return out
